# revision 1
# baseline (speedup 1.0000x reference)
"""Trainium2 Bass kernel for nn_Cholesky_from_z.

Reference computation (per batch sample b, n=128):
    s starts at 0 per row i; for column j: col = z[i,j]*sqrt(1-s) below diag,
    sqrt(1-s) on diag, 0 above; s += col^2.
Closed form: 1-s at (row i, col j) = prod_{k<j} (1 - z[i,k]^2), so
    L[i,j] = z[i,j] * sqrt(prod_{k<j}(1-z[i,k]^2))   (j < i)
    L[i,i] =          sqrt(prod_{k<i}(1-z[i,k]^2))
i.e. an exclusive cumulative product of (1-z^2) along each matrix row,
independent per row and per sample.

Device mapping: each sample's strictly-lower entries are packed row-major with
a 1.0 sentinel appended after each row (the "diagonal slot"), 8256 slots total.
One leading 1.0 column is prepended so every chunk can read one element back
for the shift.  On device, per [128 samples x chunk] tile:
    u = Square(z)                      (ACT)
    a = 1 - u, shifted one slot right  (DVE tensor_scalar)
        -> a = 0 exactly at each row-start slot (previous slot is the 1.0
           sentinel), which marks segment boundaries for free
    b = (a == 0) ? 1 : 0               (DVE, computed once; reused)
    d = scan: state = a*state + b      (DVE tensor_tensor_scan = segmented
                                        exclusive cumprod, carried across
                                        chunks via `initial`)
    q = Sqrt(d)                        (ACT)
    out = z * q                        (DVE)  [diag slot: 1 * q = q]
Batch dim (2048) is sharded 256 samples per core across 8 cores; each core
processes 2 partition-blocks of 128 samples.
"""

import sys

if "/opt/trn_rl_repo" not in sys.path:
    sys.path.insert(0, "/opt/trn_rl_repo")

import numpy as np

B = 2048
N = 128
NZ = N * (N - 1) // 2          # 8128 strictly-lower entries
PACKED = NZ + N                # 8256 slots incl. diagonal sentinels
NCORES = 8
B_CORE = B // NCORES           # 256
# ramp chunk schedule: small first/last chunks shorten pipeline fill/drain
CHUNKS = [1376, 2752, 2752, 1376]          # sums to PACKED (8256)
CHUNK_OFF = [0, 1376, 4128, 6880]
CMAX = max(CHUNKS)

# --- host-side index maps ---------------------------------------------------
# packed slot order: row i -> [z[i,0..i-1], diag_i]; row-start offset i(i+1)/2
_rows, _cols = np.tril_indices(N, -1)                  # row-major strict lower
_strict_slots = (_rows * (_rows + 1) // 2 + _cols).astype(np.int64)
_diag_slots = (np.arange(N) * (np.arange(N) + 1) // 2 + np.arange(N)).astype(np.int64)
# position of each packed slot in the dense [128,128] row-major output
_out_pos = np.empty(PACKED, np.int64)
_out_pos[_strict_slots] = _rows * N + _cols
_out_pos[_diag_slots] = np.arange(N) * N + np.arange(N)

_prog_cache = {}


def _build_program():
    import concourse.bacc as bacc
    import concourse.mybir as mybir
    from concourse.tile import TileContext

    f32 = mybir.dt.float32
    Alu = mybir.AluOpType
    Act = mybir.ActivationFunctionType

    nc = bacc.Bacc("TRN2", target_bir_lowering=False, debug=False,
                   num_devices=NCORES)
    zp = nc.dram_tensor("zp", [B_CORE, PACKED + 1], f32,
                        kind="ExternalInput").ap()
    lp = nc.dram_tensor("lp", [B_CORE, PACKED], f32,
                        kind="ExternalOutput").ap()

    NBLK = B_CORE // 128
    with TileContext(nc) as tc:
        with (
            tc.tile_pool(name="io", bufs=3) as io_pool,
            tc.tile_pool(name="up", bufs=2) as u_pool,
            tc.tile_pool(name="wq", bufs=2) as wq_pool,
            tc.tile_pool(name="dp", bufs=2) as dpool,
            tc.tile_pool(name="lt", bufs=3) as lt_pool,
            tc.tile_pool(name="bpool", bufs=1) as bpool,
        ):
            # DVE: scan + final multiply + b only; ACT: Square, 1-u, Sqrt.
            # GPSIMD untouched: concurrent GPSIMD inflates every engine ~20%.
            btiles = {}
            for blk in range(NBLK):
                r0 = blk * 128
                dprev = None
                for ch, (C, c0) in enumerate(zip(CHUNKS, CHUNK_OFF)):
                    zt = io_pool.tile([128, CMAX + 1], f32, tag="zt")
                    nc.sync.dma_start(out=zt[:, 0:C + 1],
                                      in_=zp[r0:r0 + 128, c0:c0 + C + 1])

                    # b = (prev slot == 1.0): boundary iff previous slot is
                    # the 1.0 diagonal sentinel (data slots are |z|<0.9).
                    # depends only on zt -> off the a/scan critical path.
                    if blk == 0:
                        bt = bpool.tile([128, CMAX], f32, tag=f"b{ch}")
                        nc.vector.tensor_scalar(bt[:, 0:C], zt[:, 0:C], 1.0,
                                                None, Alu.is_equal)
                        btiles[ch] = bt
                    bt = btiles[ch]

                    u = u_pool.tile([128, CMAX + 1], f32, tag="u")
                    nc.scalar.activation(u[:, 0:C + 1], zt[:, 0:C + 1],
                                         Act.Square)

                    # a[t] = 1 - u[t]  (u[t] already the shifted square)
                    a = wq_pool.tile([128, CMAX], f32, tag="w")
                    nc.scalar.activation(a[:, 0:C], u[:, 0:C],
                                         Act.Copy, bias=1.0, scale=-1.0)

                    d = dpool.tile([128, CMAX], f32, tag="d")
                    init = 1.0 if ch == 0 else dprev[0][:, dprev[1] - 1:dprev[1]]
                    nc.vector.tensor_tensor_scan(d[:, 0:C], a[:, 0:C],
                                                 bt[:, 0:C], init,
                                                 Alu.mult, Alu.add)
                    dprev = (d, C)

                    # a dead after scan; q reuses its slots (same tag)
                    q = wq_pool.tile([128, CMAX], f32, tag="w")
                    nc.scalar.activation(q[:, 0:C], d[:, 0:C], Act.Sqrt)

                    lt = lt_pool.tile([128, CMAX], f32, tag="lt")
                    nc.vector.tensor_mul(lt[:, 0:C], zt[:, 1:C + 1], q[:, 0:C])
                    nc.sync.dma_start(out=lp[r0:r0 + 128, c0:c0 + C],
                                      in_=lt[:, 0:C])
    nc.compile()
    return nc


def _get_program():
    if "nc" not in _prog_cache:
        _prog_cache["nc"] = _build_program()
    return _prog_cache["nc"]


def _run(in_maps, **kw):
    from concourse.bass_utils import run_bass_kernel_spmd

    nc = _get_program()
    return run_bass_kernel_spmd(nc, in_maps, list(range(NCORES)), **kw)


def kernel(inputs: np.ndarray, _return_raw=False, **run_kw) -> np.ndarray:
    assert inputs.shape == (B, NZ), inputs.shape
    zvec = np.ascontiguousarray(inputs, dtype=np.float32)

    # pack: one leading 1.0 column (shift sentinel) + per-row
    # [z..., 1.0 sentinel]
    zp = np.ones((B, PACKED + 1), np.float32)
    zp[:, 1 + _strict_slots] = zvec

    in_maps = [
        {"zp": np.ascontiguousarray(zp[c * B_CORE:(c + 1) * B_CORE])}
        for c in range(NCORES)
    ]
    res = _run(in_maps, **run_kw)

    lp = np.empty((B, PACKED), np.float32)
    for c in range(NCORES):
        lp[c * B_CORE:(c + 1) * B_CORE] = res.results[c]["lp"]

    out = np.zeros((B, N * N), np.float32)
    out[:, _out_pos] = lp
    out = out.reshape(B, N, N)
    if _return_raw:
        return out, res
    return out



# revision 2
# speedup vs baseline: 1.8051x; 1.8051x over previous
"""Trainium2 Bass kernel for nn_Cholesky_from_z.

Reference computation (per batch sample b, n=128):
    s starts at 0 per row i; for column j: col = z[i,j]*sqrt(1-s) below diag,
    sqrt(1-s) on diag, 0 above; s += col^2.
Closed form: 1-s at (row i, col j) = prod_{k<j} (1 - z[i,k]^2), so
    L[i,j] = z[i,j] * prod_{k<j} sqrt(1-z[i,k]^2)   (j < i)
    L[i,i] =          prod_{k<i} sqrt(1-z[i,k]^2)
i.e. an exclusive cumulative product of g = sqrt(1-z^2) along each matrix
row, independent per row and per sample.

Device mapping: each sample's strictly-lower entries are packed row-major
with a 1.0 sentinel appended after each row (the "diagonal slot"), 8256
slots total, fp16.  One leading 1.0 column is prepended so every chunk can
read one element back for the shift.  Per [128 samples x chunk] tile:
    u = Square(ztA)                 (ACT)  ztA = shifted z window
    g = Sqrt(-u + 1)                (ACT)  = sqrt(1-z^2), shifted
        -> g = 0 exactly at each row-start slot (previous slot is the 1.0
           sentinel), which marks segment boundaries for free
    b = (ztA == 1.0) ? 1 : 0        (DVE tensor_scalar 4x, computed once)
    q = scan: state = g*state + b   (DVE tensor_tensor_scan = segmented
                                     exclusive cumprod-of-sqrt, carried
                                     across chunks via `initial`; fp16 out)
    out = ztB * q                   (DVE fp16 2x_1p)  [diag slot: 1*q = q]
ztB is a second, 4B-aligned DMA of the unshifted z chunk: the 2x_1p DVE
mode requires 4-byte-aligned operands, which the +1-shifted view of ztA
cannot provide in fp16.  Input and output travel as fp16 (validated
relfro ~4e-4 vs the 2e-2 budget); the scan input g stays f32 so the
running product accumulates no rounding, and the scan state is fp32 in HW.
Batch dim (2048) is sharded 256 samples per core across 8 cores; the two
128-sample blocks are interleaved chunk-wise so the per-block scan carry
chains overlap on the engines.
"""

import sys

if "/opt/trn_rl_repo" not in sys.path:
    sys.path.insert(0, "/opt/trn_rl_repo")

import numpy as np

B = 2048
N = 128
NZ = N * (N - 1) // 2          # 8128 strictly-lower entries
PACKED = NZ + N                # 8256 slots incl. diagonal sentinels
NCORES = 8
B_CORE = B // NCORES           # 256
# ramp chunk schedule: small first/last chunks shorten pipeline fill/drain
CHUNKS = [1376, 2752, 2752, 1376]          # sums to PACKED (8256)
CHUNK_OFF = [0, 1376, 4128, 6880]
CMAX = max(CHUNKS)
MULT_ENGINE = "vector"         # "vector" | "gpsimd"

# --- host-side index maps ---------------------------------------------------
# packed slot order: row i -> [z[i,0..i-1], diag_i]; row-start offset i(i+1)/2
_rows, _cols = np.tril_indices(N, -1)                  # row-major strict lower
_strict_slots = (_rows * (_rows + 1) // 2 + _cols).astype(np.int64)
_diag_slots = (np.arange(N) * (np.arange(N) + 1) // 2 + np.arange(N)).astype(np.int64)
# position of each packed slot in the dense [128,128] row-major output
_out_pos = np.empty(PACKED, np.int64)
_out_pos[_strict_slots] = _rows * N + _cols
_out_pos[_diag_slots] = np.arange(N) * N + np.arange(N)

_prog_cache = {}


def _build_program():
    import concourse.bacc as bacc
    import concourse.mybir as mybir
    from concourse.tile import TileContext

    f32 = mybir.dt.float32
    f16 = mybir.dt.float16
    Alu = mybir.AluOpType
    Act = mybir.ActivationFunctionType

    nc = bacc.Bacc("TRN2", target_bir_lowering=False, debug=False,
                   num_devices=NCORES)
    zp = nc.dram_tensor("zp", [B_CORE, PACKED + 1], f16,
                        kind="ExternalInput").ap()
    lp = nc.dram_tensor("lp", [B_CORE, PACKED], f16,
                        kind="ExternalOutput").ap()

    NBLK = B_CORE // 128
    mult_eng = {"vector": "vector", "gpsimd": "gpsimd"}[MULT_ENGINE]
    with TileContext(nc) as tc:
        with (
            tc.tile_pool(name="ioA", bufs=3) as ioA_pool,
            tc.tile_pool(name="ioB", bufs=3) as ioB_pool,
            tc.tile_pool(name="up", bufs=2) as u_pool,
            tc.tile_pool(name="gp", bufs=3) as g_pool,
            tc.tile_pool(name="qp", bufs=3) as q_pool,
            tc.tile_pool(name="lt", bufs=3) as lt_pool,
            tc.tile_pool(name="bpool", bufs=1) as bpool,
            tc.tile_pool(name="warm", bufs=1) as warm_pool,
        ):
            # Warm the ACT function table before the first DMA lands: a tiny
            # Sqrt triggers the (combined Square/Sqrt/Copy) table load so no
            # ACT_TABLE_LOAD sits on the first chunk's critical path.
            wt = warm_pool.tile([128, 4], f32, tag="warm")
            nc.vector.memset(wt[:, 0:4], 0.0)
            nc.scalar.activation(wt[:, 0:2], wt[:, 2:4], Act.Sqrt)

            btiles = {}
            qprev = {}
            for ch, (C, c0) in enumerate(zip(CHUNKS, CHUNK_OFF)):
                for blk in range(NBLK):
                    r0 = blk * 128
                    # shifted window (covers packed[c0-1 .. c0+C-1])
                    ztA = ioA_pool.tile([128, CMAX + 1], f16, tag="ztA")
                    nc.sync.dma_start(out=ztA[:, 0:C + 1],
                                      in_=zp[r0:r0 + 128, c0:c0 + C + 1])
                    # aligned unshifted window (packed[c0 .. c0+C-1])
                    ztB = ioB_pool.tile([128, CMAX], f16, tag="ztB")
                    nc.sync.dma_start(out=ztB[:, 0:C],
                                      in_=zp[r0:r0 + 128, c0 + 1:c0 + C + 1])

                    # b = (shifted z == 1.0): boundary iff previous slot is
                    # the 1.0 diagonal sentinel (data slots are |z|<0.901).
                    if blk == 0:
                        bt = bpool.tile([128, CMAX], f16, tag=f"b{ch}")
                        nc.vector.tensor_scalar(bt[:, 0:C], ztA[:, 0:C], 1.0,
                                                None, Alu.is_equal)
                        btiles[ch] = bt
                    bt = btiles[ch]

                    u = u_pool.tile([128, CMAX], f32, tag="u")
                    nc.scalar.activation(u[:, 0:C], ztA[:, 0:C], Act.Square)

                    # g = sqrt(1 - u)  (shifted, zero at row starts)
                    g = g_pool.tile([128, CMAX], f32, tag="g")
                    nc.scalar.activation(g[:, 0:C], u[:, 0:C], Act.Sqrt,
                                         bias=1.0, scale=-1.0)

                    q = q_pool.tile([128, CMAX], f16, tag="q")
                    if ch == 0:
                        init = 1.0
                    else:
                        qp_t, qp_c = qprev[blk]
                        init = qp_t[:, qp_c - 1:qp_c]
                    nc.vector.tensor_tensor_scan(q[:, 0:C], g[:, 0:C],
                                                 bt[:, 0:C], init,
                                                 Alu.mult, Alu.add)
                    qprev[blk] = (q, C)

                    lt = lt_pool.tile([128, CMAX], f16, tag="lt")
                    getattr(nc, mult_eng).tensor_mul(lt[:, 0:C], ztB[:, 0:C],
                                                     q[:, 0:C])
                    nc.sync.dma_start(out=lp[r0:r0 + 128, c0:c0 + C],
                                      in_=lt[:, 0:C])
    nc.compile()
    return nc


def _get_program():
    if "nc" not in _prog_cache:
        _prog_cache["nc"] = _build_program()
    return _prog_cache["nc"]


def _run(in_maps, **kw):
    from concourse.bass_utils import run_bass_kernel_spmd

    nc = _get_program()
    return run_bass_kernel_spmd(nc, in_maps, list(range(NCORES)), **kw)


def kernel(inputs: np.ndarray, _return_raw=False, **run_kw) -> np.ndarray:
    assert inputs.shape == (B, NZ), inputs.shape
    zvec = np.ascontiguousarray(inputs, dtype=np.float32)

    # pack: one leading 1.0 column (shift sentinel) + per-row
    # [z..., 1.0 sentinel], fp16
    zp = np.ones((B, PACKED + 1), np.float16)
    zp[:, 1 + _strict_slots] = zvec.astype(np.float16)

    in_maps = [
        {"zp": np.ascontiguousarray(zp[c * B_CORE:(c + 1) * B_CORE])}
        for c in range(NCORES)
    ]
    res = _run(in_maps, **run_kw)

    lp = np.empty((B, PACKED), np.float16)
    for c in range(NCORES):
        lp[c * B_CORE:(c + 1) * B_CORE] = res.results[c]["lp"]

    out = np.zeros((B, N * N), np.float32)
    out[:, _out_pos] = lp.astype(np.float32)
    out = out.reshape(B, N, N)
    if _return_raw:
        return out, res
    return out
